# revision 3
# baseline (speedup 1.0000x reference)
"""Multi-head self-attention (B=2, S=2048, D=768, H=12) on 8 trn2 NeuronCores.

Sharding: core c = 4*b + g handles batch b and head-group g (3 heads = 192 of
the 768 model dims). Weights are column-split (wq/wk/wv) and row-split (wo);
each core emits a partial (2048, 768) output; the host sums the 4 group
partials per batch and adds bo.

Device-side dataflow is transpose-free: inputs arrive pre-transposed (D, S),
so projections produce Q^T/K^T in (head_dim, S) layout which feeds the
logits matmul directly; softmax is computed as exp(logits/8) without
max-subtraction (logits are ~N(0,1), exp cannot overflow) with denominators
obtained from a ones-column appended to V in the context matmul; the context
comes out transposed (dims, S), which is exactly the stationary operand the
output projection needs.
"""
import numpy as np
from contextlib import ExitStack

import concourse.bacc as bacc
import concourse.mybir as mybir
import concourse.tile as tile
from concourse import bass_utils

# Problem shape (hardcoded per contract).
B, S, D, H, DH = 2, 2048, 768, 12, 64
NCORES = 8
NG = 4            # head groups
HG = H // NG      # heads per group (3)
G = HG * DH       # model dims per group (192)
SC = 512          # query-chunk length
NQ = S // SC      # 4 chunks
KB = 128          # key-block length
NKB = S // KB     # 16 blocks
KT6 = D // 128    # 6 contraction tiles for the projections
SEG = DH + 1      # V segment width per head: 64 V columns + 1 ones column
FP32 = mybir.dt.float32

AF = mybir.ActivationFunctionType
ALU = mybir.AluOpType

_CACHE: dict = {}


def _build():
    nc = bacc.Bacc("TRN2", target_bir_lowering=False, debug=False)

    qT = nc.dram_tensor("qT", [D, S], FP32, kind="ExternalInput")
    kT = nc.dram_tensor("kT", [D, S], FP32, kind="ExternalInput")
    vT = nc.dram_tensor("vT", [D, S], FP32, kind="ExternalInput")
    wq = nc.dram_tensor("wq", [D, G], FP32, kind="ExternalInput")
    wk = nc.dram_tensor("wk", [D, G], FP32, kind="ExternalInput")
    wv = nc.dram_tensor("wv", [D, G], FP32, kind="ExternalInput")
    wo = nc.dram_tensor("wo", [G, D], FP32, kind="ExternalInput")
    bq = nc.dram_tensor("bq", [G, 1], FP32, kind="ExternalInput")
    bk = nc.dram_tensor("bk", [G, 1], FP32, kind="ExternalInput")
    bv = nc.dram_tensor("bv", [1, G], FP32, kind="ExternalInput")
    yp = nc.dram_tensor("yp", [S, D], FP32, kind="ExternalOutput")

    with tile.TileContext(nc) as tc, ExitStack() as ctx:
        const = ctx.enter_context(tc.tile_pool(name="const", bufs=1))
        xin = ctx.enter_context(tc.tile_pool(name="xin", bufs=2))
        qtp = ctx.enter_context(tc.tile_pool(name="qtp", bufs=2))
        ppool = ctx.enter_context(tc.tile_pool(name="ppool", bufs=2))
        ctxp = ctx.enter_context(tc.tile_pool(name="ctxp", bufs=2))
        ypool = ctx.enter_context(tc.tile_pool(name="ypool", bufs=2))
        den = ctx.enter_context(tc.tile_pool(name="den", bufs=3))
        ps_proj = ctx.enter_context(tc.tile_pool(name="ps_proj", bufs=3, space="PSUM"))
        ps_log = ctx.enter_context(tc.tile_pool(name="ps_log", bufs=3, space="PSUM"))
        ps_ctx = ctx.enter_context(tc.tile_pool(name="ps_ctx", bufs=2, space="PSUM"))

        # ---- constants / weights ------------------------------------------
        wq_sb = const.tile([128, KT6, G], FP32)
        nc.sync.dma_start(wq_sb[:], qT_rearr(wq))
        wk_sb = const.tile([128, KT6, G], FP32)
        nc.sync.dma_start(wk_sb[:], qT_rearr(wk))
        wv_sb = const.tile([128, KT6, G], FP32)
        nc.sync.dma_start(wv_sb[:], qT_rearr(wv))
        wo_sb0 = const.tile([128, D], FP32)
        nc.sync.dma_start(wo_sb0[:], wo.ap()[0:128, :])
        wo_sb1 = const.tile([64, D], FP32)
        nc.sync.dma_start(wo_sb1[:], wo.ap()[128:G, :])
        bq0 = const.tile([128, 1], FP32)
        nc.sync.dma_start(bq0[:], bq.ap()[0:128, :])
        bq1 = const.tile([64, 1], FP32)
        nc.sync.dma_start(bq1[:], bq.ap()[128:G, :])
        bk0 = const.tile([128, 1], FP32)
        nc.sync.dma_start(bk0[:], bk.ap()[0:128, :])
        bk1 = const.tile([64, 1], FP32)
        nc.sync.dma_start(bk1[:], bk.ap()[128:G, :])
        bv_sb = const.tile([1, G], FP32)
        nc.sync.dma_start(bv_sb[:], bv.ap()[:, :])
        ones_row = const.tile([1, 128], FP32)
        nc.vector.memset(ones_row[:], 1.0)

        KT0 = const.tile([128, S], FP32)   # K^T heads 0,1 of the group
        KT1 = const.tile([64, S], FP32)    # K^T head 2
        Vg = const.tile([128, NKB, HG * SEG], FP32)  # V blocks + ones columns
        nc.vector.memset(Vg[:], 1.0)

        mblocks = ((128, 0), (64, 128))  # (rows, row-offset) of the 192 dims

        # ---- phase 1: K^T = wk^T @ kT + bk -------------------------------
        for sc_i in range(NQ):
            kx = xin.tile([128, KT6, SC], FP32, tag="xin")
            nc.sync.dma_start(
                kx[:],
                kT.ap()[:, sc_i * SC:(sc_i + 1) * SC].rearrange(
                    "(t p) s -> p t s", p=128
                ),
            )
            for m, (mp, mo) in enumerate(mblocks):
                ps = ps_proj.tile([128, SC], FP32, tag="pp")
                for t in range(KT6):
                    nc.tensor.matmul(
                        ps[:mp, :], wk_sb[:, t, mo:mo + mp], kx[:, t, :],
                        start=(t == 0), stop=(t == KT6 - 1),
                    )
                dst = KT0 if m == 0 else KT1
                bias = bk0 if m == 0 else bk1
                nc.vector.tensor_scalar_add(
                    dst[:mp, sc_i * SC:(sc_i + 1) * SC], ps[:mp, :], bias[:mp, :]
                )

        # ---- phase 2: V = vT^T @ wv + bv (natural layout, + ones cols) ---
        for sb in range(NKB):
            vx = xin.tile([128, KT6, KB], FP32, tag="xin")
            nc.sync.dma_start(
                vx[:],
                vT.ap()[:, sb * KB:(sb + 1) * KB].rearrange(
                    "(t p) s -> p t s", p=128
                ),
            )
            ps = ps_proj.tile([128, G], FP32, tag="pp")
            for t in range(KT6):
                nc.tensor.matmul(
                    ps[:], vx[:, t, :], wv_sb[:, t, :],
                    start=(t == 0), stop=False,
                )
            nc.tensor.matmul(ps[:], ones_row[:], bv_sb[:], start=False, stop=True)
            for h in range(HG):
                nc.vector.tensor_copy(
                    Vg[:, sb, h * SEG:h * SEG + DH], ps[:, h * DH:(h + 1) * DH]
                )

        # ---- phase 3: per q-chunk attention + output projection ----------
        for qc in range(NQ):
            qx = xin.tile([128, KT6, SC], FP32, tag="xin")
            nc.sync.dma_start(
                qx[:],
                qT.ap()[:, qc * SC:(qc + 1) * SC].rearrange(
                    "(t p) s -> p t s", p=128
                ),
            )
            QT0 = qtp.tile([128, SC], FP32, tag="qt0")
            QT1 = qtp.tile([64, SC], FP32, tag="qt1")
            for m, (mp, mo) in enumerate(mblocks):
                ps = ps_proj.tile([128, SC], FP32, tag="pp")
                for t in range(KT6):
                    nc.tensor.matmul(
                        ps[:mp, :], wq_sb[:, t, mo:mo + mp], qx[:, t, :],
                        start=(t == 0), stop=(t == KT6 - 1),
                    )
                dst = QT0 if m == 0 else QT1
                bias = bq0 if m == 0 else bq1
                nc.vector.tensor_scalar_add(dst[:mp, :], ps[:mp, :], bias[:mp, :])

            ctxT0 = ctxp.tile([128, SC], FP32, tag="c0")
            ctxT1 = ctxp.tile([64, SC], FP32, tag="c1")
            P = ppool.tile([128, NKB, SC], FP32, tag="P")
            for h in range(HG):
                if h < 2:
                    kt_t, koff = KT0, 64 * h
                    qt_t, qoff = QT0, 64 * h
                else:
                    kt_t, koff = KT1, 0
                    qt_t, qoff = QT1, 0
                # logits^T (key-major) then P = exp(logits/8)
                for kb in range(NKB):
                    pl = ps_log.tile([128, SC], FP32, tag="pl")
                    nc.tensor.matmul(
                        pl[:],
                        kt_t[koff:koff + DH, kb * KB:(kb + 1) * KB],
                        qt_t[qoff:qoff + DH, :],
                        start=True, stop=True,
                    )
                    nc.scalar.activation(
                        P[:, kb, :], pl[:], AF.Exp, scale=1.0 / np.sqrt(DH)
                    )
                # ctx^T (+ denominator row) = [V | 1]^T @ P
                pc = ps_ctx.tile([SEG, SC], FP32, tag="pc")
                for kb in range(NKB):
                    nc.tensor.matmul(
                        pc[:],
                        Vg[:, kb, h * SEG:(h + 1) * SEG],
                        P[:, kb, :],
                        start=(kb == 0), stop=(kb == NKB - 1),
                    )
                rden = den.tile([1, SC], FP32, tag="rden")
                nc.vector.reciprocal(rden[:], pc[DH:SEG, :])
                rbc = den.tile([64, SC], FP32, tag="rbc")
                nc.sync.dma_start(
                    rbc[:], rden[:, None, :].to_broadcast((1, 64, SC))
                )
                if h < 2:
                    cdst = ctxT0[64 * h:64 * h + 64, :]
                else:
                    cdst = ctxT1[0:64, :]
                nc.vector.tensor_tensor(
                    cdst, pc[0:DH, :], rbc[:], ALU.mult
                )

            # output projection: yp_chunk = ctxT^T @ wo
            for half in range(2):
                Yt = ypool.tile([128, 2, D], FP32, tag="Y")
                for m in range(2):
                    sb = half * 2 + m
                    for nh in range(2):
                        py = ps_proj.tile([128, D // 2], FP32, tag="pp")
                        nc.tensor.matmul(
                            py[:],
                            ctxT0[:, sb * 128:(sb + 1) * 128],
                            wo_sb0[:, nh * (D // 2):(nh + 1) * (D // 2)],
                            start=True, stop=False,
                        )
                        nc.tensor.matmul(
                            py[:],
                            ctxT1[:, sb * 128:(sb + 1) * 128],
                            wo_sb1[:, nh * (D // 2):(nh + 1) * (D // 2)],
                            start=False, stop=True,
                        )
                        nc.vector.tensor_copy(
                            Yt[:, m, nh * (D // 2):(nh + 1) * (D // 2)], py[:]
                        )
                nc.sync.dma_start(
                    yp.ap()[
                        qc * SC + half * 256:qc * SC + (half + 1) * 256, :
                    ].rearrange("(m p) d -> p m d", p=128),
                    Yt[:],
                )

    nc.compile()
    return nc


def qT_rearr(w):
    return w.ap().rearrange("(t p) g -> p t g", p=128)


def _get_nc():
    if "nc" not in _CACHE:
        _CACHE["nc"] = _build()
    return _CACHE["nc"]


def _in_maps(v, k, q, wq, bq, wk, bk, wv, bv, wo, bo):
    f32 = lambda a: np.ascontiguousarray(np.asarray(a, dtype=np.float32))
    qTb = [f32(q[b].T) for b in range(B)]
    kTb = [f32(k[b].T) for b in range(B)]
    vTb = [f32(v[b].T) for b in range(B)]
    maps = []
    for c in range(NCORES):
        b, g = divmod(c, NG)
        cols = slice(g * G, (g + 1) * G)
        maps.append({
            "qT": qTb[b],
            "kT": kTb[b],
            "vT": vTb[b],
            "wq": f32(wq[:, cols]),
            "wk": f32(wk[:, cols]),
            "wv": f32(wv[:, cols]),
            "wo": f32(wo[cols, :]),
            "bq": f32(np.asarray(bq)[cols].reshape(G, 1)),
            "bk": f32(np.asarray(bk)[cols].reshape(G, 1)),
            "bv": f32(np.asarray(bv)[cols].reshape(1, G)),
        })
    return maps


def kernel(v, k, q, wq, bq, wk, bk, wv, bv, wo, bo, _trace=False):
    nc = _get_nc()
    in_maps = _in_maps(v, k, q, wq, bq, wk, bk, wv, bv, wo, bo)
    res = bass_utils.run_bass_kernel_spmd(
        nc, in_maps, core_ids=list(range(NCORES)), trace=_trace
    )
    bo = np.asarray(bo, dtype=np.float32)
    out = np.empty((B, S, D), dtype=np.float32)
    for b in range(B):
        acc = res.results[4 * b]["yp"].astype(np.float32)
        for g in range(1, NG):
            acc = acc + res.results[4 * b + g]["yp"]
        out[b] = acc + bo[None, :]
    if _trace:
        kernel.last_result = res
    return out


# revision 9
# speedup vs baseline: 1.5596x; 1.5596x over previous
"""Multi-head self-attention (B=2, S=2048, D=768, H=12) on 8 trn2 NeuronCores.

Sharding: core c = 4*b + g handles batch b and head-group g (3 heads = 192 of
the 768 model dims). Weights are column-split (wq/wk/wv) and row-split (wo);
each core emits a partial (2048, 768) output; the host sums the 4 group
partials per batch and adds bo.

Device-side dataflow is transpose-free: inputs arrive pre-transposed (D, S),
so projections produce Q^T/K^T in (head_dim, S) layout which feeds the
logits matmul directly; softmax is computed as exp(logits/8) without
max-subtraction (logits are ~N(0,1), exp cannot overflow) with denominators
obtained from a ones-column appended to V in the context matmul; the context
comes out transposed (dims, S), which is exactly the stationary operand the
output projection needs.

Matmul operands use float32r (single-pass ~1.4 cyc/row vs 4+ for fp32, with
~1e-4 matmul precision); accumulation stays fp32 in PSUM.
"""
import numpy as np
from contextlib import ExitStack

import concourse.bacc as bacc
import concourse.mybir as mybir
import concourse.tile as tile
from concourse import bass_utils

# Problem shape (hardcoded per contract).
B, S, D, H, DH = 2, 2048, 768, 12, 64
NCORES = 8
NG = 4            # head groups
HG = H // NG      # heads per group (3)
G = HG * DH       # model dims per group (192)
SC = 512          # query-chunk length
NQ = S // SC      # 4 chunks
KB = 128          # key-block length
NKB = S // KB     # 16 blocks
KT6 = D // 128    # 6 contraction tiles for the projections
SEG = DH + 1      # V segment width per head: 64 V columns + 1 ones column
FP32 = mybir.dt.float32
CDT = mybir.dt.float32r   # matmul-operand dtype

AF = mybir.ActivationFunctionType
ALU = mybir.AluOpType

_CACHE: dict = {}


def _build():
    nc = bacc.Bacc("TRN2", target_bir_lowering=False, debug=False)

    qT = nc.dram_tensor("qT", [D, S], CDT, kind="ExternalInput")
    kT = nc.dram_tensor("kT", [D, S], CDT, kind="ExternalInput")
    vT = nc.dram_tensor("vT", [D, S], CDT, kind="ExternalInput")
    wq = nc.dram_tensor("wq", [D, G], CDT, kind="ExternalInput")
    wk = nc.dram_tensor("wk", [D, G], CDT, kind="ExternalInput")
    wv = nc.dram_tensor("wv", [D, G], CDT, kind="ExternalInput")
    wo = nc.dram_tensor("wo", [G, D], CDT, kind="ExternalInput")
    bq = nc.dram_tensor("bq", [G, 1], FP32, kind="ExternalInput")
    bk = nc.dram_tensor("bk", [G, 1], FP32, kind="ExternalInput")
    bv = nc.dram_tensor("bv", [1, G], CDT, kind="ExternalInput")
    cones = nc.dram_tensor("cones", [1, 1], CDT, kind="ExternalInput")
    yp = nc.dram_tensor("yp", [S, D], FP32, kind="ExternalOutput")

    with tile.TileContext(nc) as tc, ExitStack() as ctx:
        const = ctx.enter_context(tc.tile_pool(name="const", bufs=1))
        xin = ctx.enter_context(tc.tile_pool(name="xin", bufs=2))
        qtp = ctx.enter_context(tc.tile_pool(name="qtp", bufs=2))
        ppool = ctx.enter_context(tc.tile_pool(name="ppool", bufs=2))
        ctxp = ctx.enter_context(tc.tile_pool(name="ctxp", bufs=2))
        ypool = ctx.enter_context(tc.tile_pool(name="ypool", bufs=2))
        den = ctx.enter_context(tc.tile_pool(name="den", bufs=3))
        ps_proj = ctx.enter_context(tc.tile_pool(name="ps_proj", bufs=3, space="PSUM"))
        ps_log = ctx.enter_context(tc.tile_pool(name="ps_log", bufs=3, space="PSUM"))
        ps_ctx = ctx.enter_context(tc.tile_pool(name="ps_ctx", bufs=2, space="PSUM"))

        def rearr6(w):
            return w.ap().rearrange("(t p) g -> p t g", p=128)

        # ---- constants / weights ------------------------------------------
        wq_sb = const.tile([128, KT6, G], CDT)
        nc.sync.dma_start(wq_sb[:], rearr6(wq))
        wk_sb = const.tile([128, KT6, G], CDT)
        nc.sync.dma_start(wk_sb[:], rearr6(wk))
        wv_sb = const.tile([128, KT6, G], CDT)
        nc.sync.dma_start(wv_sb[:], rearr6(wv))
        wo_sb0 = const.tile([128, D], CDT)
        nc.sync.dma_start(wo_sb0[:], wo.ap()[0:128, :])
        wo_sb1 = const.tile([64, D], CDT)
        nc.sync.dma_start(wo_sb1[:], wo.ap()[128:G, :])
        bq0 = const.tile([128, 1], FP32)
        nc.sync.dma_start(bq0[:], bq.ap()[0:128, :])
        bq1 = const.tile([64, 1], FP32)
        nc.sync.dma_start(bq1[:], bq.ap()[128:G, :])
        bk0 = const.tile([128, 1], FP32)
        nc.sync.dma_start(bk0[:], bk.ap()[0:128, :])
        bk1 = const.tile([64, 1], FP32)
        nc.sync.dma_start(bk1[:], bk.ap()[128:G, :])
        bv_sb = const.tile([1, G], CDT)
        nc.sync.dma_start(bv_sb[:], bv.ap()[:, :])
        ones_row = const.tile([1, 128], CDT)
        nc.sync.dma_start(
            ones_row[:], cones.ap()[:, :, None].to_broadcast((1, 128, 1))
        )

        KT0 = const.tile([128, S], CDT)   # K^T heads 0,1 of the group
        KT1 = const.tile([64, S], CDT)    # K^T head 2
        Vg = const.tile([128, NKB, HG * SEG], CDT)  # V blocks + ones columns
        for h in range(HG):
            nc.sync.dma_start(
                Vg[:, :, h * SEG + DH:h * SEG + DH + 1],
                cones.ap().to_broadcast((128, NKB, 1)),
            )

        mblocks = ((128, 0), (64, 128))  # (rows, row-offset) of the 192 dims

        # ---- phase 1: K^T = wk^T @ kT + bk -------------------------------
        for sc_i in range(NQ):
            kx = xin.tile([128, KT6, SC], CDT, tag="xin")
            nc.sync.dma_start(
                kx[:],
                kT.ap()[:, sc_i * SC:(sc_i + 1) * SC].rearrange(
                    "(t p) s -> p t s", p=128
                ),
            )
            for m, (mp, mo) in enumerate(mblocks):
                ps = ps_proj.tile([128, SC], FP32, tag="pp")
                for t in range(KT6):
                    nc.tensor.matmul(
                        ps[:mp, :], wk_sb[:, t, mo:mo + mp], kx[:, t, :],
                        start=(t == 0), stop=(t == KT6 - 1),
                    )
                dst = KT0 if m == 0 else KT1
                bias = bk0 if m == 0 else bk1
                nc.vector.tensor_scalar_add(
                    dst[:mp, sc_i * SC:(sc_i + 1) * SC], ps[:mp, :], bias[:mp, :]
                )

        # ---- phase 2: V = vT^T @ wv + bv (natural layout, + ones cols) ---
        for sb in range(NKB):
            vx = xin.tile([128, KT6, KB], CDT, tag="xin")
            nc.sync.dma_start(
                vx[:],
                vT.ap()[:, sb * KB:(sb + 1) * KB].rearrange(
                    "(t p) s -> p t s", p=128
                ),
            )
            ps = ps_proj.tile([128, G], FP32, tag="pp")
            for t in range(KT6):
                nc.tensor.matmul(
                    ps[:], vx[:, t, :], wv_sb[:, t, :],
                    start=(t == 0), stop=False,
                )
            nc.tensor.matmul(ps[:], ones_row[:], bv_sb[:], start=False, stop=True)
            for h in range(HG):
                nc.vector.tensor_copy(
                    Vg[:, sb, h * SEG:h * SEG + DH], ps[:, h * DH:(h + 1) * DH]
                )

        # ---- phase 3: per q-chunk attention + output projection ----------
        for qc in range(NQ):
            qx = xin.tile([128, KT6, SC], CDT, tag="xin")
            nc.sync.dma_start(
                qx[:],
                qT.ap()[:, qc * SC:(qc + 1) * SC].rearrange(
                    "(t p) s -> p t s", p=128
                ),
            )
            QT0 = qtp.tile([128, SC], CDT, tag="qt0")
            QT1 = qtp.tile([64, SC], CDT, tag="qt1")
            for m, (mp, mo) in enumerate(mblocks):
                ps = ps_proj.tile([128, SC], FP32, tag="pp")
                for t in range(KT6):
                    nc.tensor.matmul(
                        ps[:mp, :], wq_sb[:, t, mo:mo + mp], qx[:, t, :],
                        start=(t == 0), stop=(t == KT6 - 1),
                    )
                dst = QT0 if m == 0 else QT1
                bias = bq0 if m == 0 else bq1
                nc.vector.tensor_scalar_add(dst[:mp, :], ps[:mp, :], bias[:mp, :])

            ctxT0 = ctxp.tile([128, SC], CDT, tag="c0")
            ctxT1 = ctxp.tile([64, SC], CDT, tag="c1")
            P = ppool.tile([128, NKB, SC], CDT, tag="P")
            for h in range(HG):
                if h < 2:
                    kt_t, koff = KT0, 64 * h
                    qt_t, qoff = QT0, 64 * h
                else:
                    kt_t, koff = KT1, 0
                    qt_t, qoff = QT1, 0
                # logits^T (key-major) then P = exp(logits/8)
                for kb in range(NKB):
                    pl = ps_log.tile([128, SC], FP32, tag="pl")
                    nc.tensor.matmul(
                        pl[:],
                        kt_t[koff:koff + DH, kb * KB:(kb + 1) * KB],
                        qt_t[qoff:qoff + DH, :],
                        start=True, stop=True,
                    )
                    nc.scalar.activation(
                        P[:, kb, :], pl[:], AF.Exp, scale=1.0 / np.sqrt(DH)
                    )
                # ctx^T (+ denominator row) = [V | 1]^T @ P
                pc = ps_ctx.tile([SEG, SC], FP32, tag="pc")
                for kb in range(NKB):
                    nc.tensor.matmul(
                        pc[:],
                        Vg[:, kb, h * SEG:(h + 1) * SEG],
                        P[:, kb, :],
                        start=(kb == 0), stop=(kb == NKB - 1),
                    )
                # normalization: recip of the denominator row, broadcast to
                # 64 partitions. The (1, SC) row is spread over 64 partitions
                # first so the iterative-divide reciprocal runs 64-wide.
                cu = den.tile([SEG, SC], FP32, tag="cu")
                nc.vector.tensor_copy(cu[:], pc[:])
                d8 = den.tile([64, SC // 64], FP32, tag="d8")
                nc.sync.dma_start(
                    d8[:],
                    cu[DH:SEG, :].rearrange("o (p f) -> o p f", p=64),
                )
                r8 = den.tile([64, SC // 64], FP32, tag="r8")
                nc.vector.reciprocal(r8[:], d8[:])
                rrow = den.tile([1, SC], FP32, tag="rrow")
                nc.sync.dma_start(
                    rrow[:].rearrange("o (p f) -> o p f", p=64), r8[:]
                )
                rbc = den.tile([64, SC], FP32, tag="rbc")
                nc.sync.dma_start(
                    rbc[:], rrow[:, None, :].to_broadcast((1, 64, SC))
                )
                if h < 2:
                    cdst = ctxT0[64 * h:64 * h + 64, :]
                else:
                    cdst = ctxT1[0:64, :]
                nc.vector.tensor_tensor(cdst, cu[0:DH, :], rbc[:], ALU.mult)

            # output projection: yp_chunk = ctxT^T @ wo
            for half in range(2):
                Yt = ypool.tile([128, 2, D], FP32, tag="Y")
                for m in range(2):
                    sb = half * 2 + m
                    for nh in range(2):
                        py = ps_proj.tile([128, D // 2], FP32, tag="pp")
                        nc.tensor.matmul(
                            py[:],
                            ctxT0[:, sb * 128:(sb + 1) * 128],
                            wo_sb0[:, nh * (D // 2):(nh + 1) * (D // 2)],
                            start=True, stop=False,
                        )
                        nc.tensor.matmul(
                            py[:],
                            ctxT1[:, sb * 128:(sb + 1) * 128],
                            wo_sb1[:, nh * (D // 2):(nh + 1) * (D // 2)],
                            start=False, stop=True,
                        )
                        nc.vector.tensor_copy(
                            Yt[:, m, nh * (D // 2):(nh + 1) * (D // 2)], py[:]
                        )
                nc.sync.dma_start(
                    yp.ap()[
                        qc * SC + half * 256:qc * SC + (half + 1) * 256, :
                    ].rearrange("(m p) d -> p m d", p=128),
                    Yt[:],
                )

    nc.compile()
    return nc


def _get_nc():
    if "nc" not in _CACHE:
        _CACHE["nc"] = _build()
    return _CACHE["nc"]


def _in_maps(v, k, q, wq, bq, wk, bk, wv, bv, wo, bo):
    f32 = lambda a: np.ascontiguousarray(np.asarray(a, dtype=np.float32))
    qTb = [f32(q[b].T) for b in range(B)]
    kTb = [f32(k[b].T) for b in range(B)]
    vTb = [f32(v[b].T) for b in range(B)]
    maps = []
    for c in range(NCORES):
        b, g = divmod(c, NG)
        cols = slice(g * G, (g + 1) * G)
        maps.append({
            "qT": qTb[b],
            "kT": kTb[b],
            "vT": vTb[b],
            "wq": f32(wq[:, cols]),
            "wk": f32(wk[:, cols]),
            "wv": f32(wv[:, cols]),
            "wo": f32(wo[cols, :]),
            "bq": f32(np.asarray(bq)[cols].reshape(G, 1)),
            "bk": f32(np.asarray(bk)[cols].reshape(G, 1)),
            "bv": f32(np.asarray(bv)[cols].reshape(1, G)),
            "cones": np.ones((1, 1), dtype=np.float32),
        })
    return maps


def kernel(v, k, q, wq, bq, wk, bk, wv, bv, wo, bo, _trace=False):
    nc = _get_nc()
    in_maps = _in_maps(v, k, q, wq, bq, wk, bk, wv, bv, wo, bo)
    res = bass_utils.run_bass_kernel_spmd(
        nc, in_maps, core_ids=list(range(NCORES)), trace=_trace
    )
    bo = np.asarray(bo, dtype=np.float32)
    out = np.empty((B, S, D), dtype=np.float32)
    for b in range(B):
        acc = res.results[4 * b]["yp"].astype(np.float32)
        for g in range(1, NG):
            acc = acc + res.results[4 * b + g]["yp"]
        out[b] = acc + bo[None, :]
    if _trace:
        kernel.last_result = res
    return out
